# revision 3
# baseline (speedup 1.0000x reference)
"""Trainium2 Bass kernel for nn_BitLayer.

Reference computation:
    x: (B=32, D=512, 1, S=64) int32 bits {0,1}
    kernel: (D=512, O=128, S=64) int32 bits {0,1}
    out[b, o, s] = (sum_d x[b,d,0,s] & kernel[d,o,s]) > 0     -> int32

Since the values are bits, AND == multiply, so for each bit position s this
is a (B x D) @ (D x O) matmul followed by a >0 threshold. The 64 bit
positions are fully independent, so we shard S across the 8 cores (8 bit
positions per core) — both inputs and the output shard along S, no
collectives needed.

Per core (S_loc = 8 bit positions):
  - host casts the {0,1} int32 bits to fp8_e4m3 (exact) and lays them out as
      k8: [128, S_loc*4*128] fp8   k8[p, (s*4+ch)*128 + o] = kernel[ch*128+p, o, s]
      x8: [128, S_loc*4*32]  fp8   x8[p, (s*4+ch)*32  + b] = x[b, ch*128+p, 0, s]
  - for each s: 4 accumulating PE matmuls over the D=512 contraction
      psum[o, b] += k8_chunk.T @ x8_chunk   (fp32 accumulate, sums <= 512: exact)
  - DVE threshold: out[o, s*32+b] = (psum > 0) as int32
  - one DMA out: o32 [128, S_loc*32] int32
"""

import numpy as np
import ml_dtypes

B, D, O, S = 32, 512, 128, 64
NCORES = 8
SL = S // NCORES          # bit positions per core = 8
P = 128                   # partition dim / contraction tile
CH = D // P               # contraction chunks = 4
F8NP = ml_dtypes.float8_e4m3

TRACE = False             # test harness can flip this for profiling
LAST = None               # last BassKernelResults (for the test harness)

_NC = None                # cached Bass module (compile once)


def _build():
    import concourse.mybir as mybir
    import concourse.tile as tile
    from concourse import bacc

    nc = bacc.Bacc(None, target_bir_lowering=False)
    f8 = mybir.dt.float8e4

    xd = nc.dram_tensor("x8", [P, SL * CH * B], f8, kind="ExternalInput")
    kd = nc.dram_tensor("k8", [P, SL * CH * O], f8, kind="ExternalInput")
    od = nc.dram_tensor("o32", [P, SL * B], mybir.dt.int32, kind="ExternalOutput")

    with tile.TileContext(nc) as tc:
        with (
            tc.tile_pool(name="inp", bufs=1) as pool,
            tc.tile_pool(name="ps", bufs=4, space="PSUM") as psum,
            tc.tile_pool(name="outp", bufs=1) as opool,
        ):
            xt = pool.tile([P, SL * CH * B], f8)
            kt = pool.tile([P, SL * CH * O], f8)
            nc.sync.dma_start(xt[:], xd[:])
            nc.sync.dma_start(kt[:], kd[:])

            ot = opool.tile([P, SL * B], mybir.dt.int32)
            for s in range(SL):
                ps = psum.tile([P, B], mybir.dt.float32)
                for ch in range(CH):
                    i = s * CH + ch
                    nc.tensor.matmul(
                        ps[:],
                        kt[:, i * O:(i + 1) * O],   # stationary lhsT [d, o]
                        xt[:, i * B:(i + 1) * B],   # moving rhs   [d, b]
                        start=(ch == 0),
                        stop=(ch == CH - 1),
                    )
                nc.vector.tensor_scalar(
                    ot[:, s * B:(s + 1) * B], ps[:], 0.0, None,
                    mybir.AluOpType.is_gt,
                )
            nc.sync.dma_start(od[:], ot[:])
    nc.compile()
    return nc


def kernel(x: np.ndarray, kernel: np.ndarray) -> np.ndarray:
    global _NC, LAST
    from concourse.bass_utils import run_bass_kernel_spmd

    if _NC is None:
        _NC = _build()

    # ---- host-side shard + layout (values are {0,1}: fp8 cast is exact) ----
    # x: (B, D, 1, S) -> (S, D, B) -> per core [128, SL*CH*B]
    xr = np.ascontiguousarray(
        x.reshape(B, D, S).astype(F8NP).transpose(2, 1, 0)
    ).reshape(NCORES, SL, CH, P, B).transpose(0, 3, 1, 2, 4)
    # kernel: (D, O, S) -> (S, D, O) -> per core [128, SL*CH*O]
    kr = np.ascontiguousarray(
        kernel.astype(F8NP).transpose(2, 0, 1)
    ).reshape(NCORES, SL, CH, P, O).transpose(0, 3, 1, 2, 4)

    in_maps = [
        {
            "x8": np.ascontiguousarray(xr[c]).reshape(P, SL * CH * B),
            "k8": np.ascontiguousarray(kr[c]).reshape(P, SL * CH * O),
        }
        for c in range(NCORES)
    ]

    LAST = run_bass_kernel_spmd(
        _NC, in_maps, core_ids=list(range(NCORES)), trace=TRACE
    )

    # ---- gather: per-core o32 [128, SL*32] = (o, s, b) -> (B, O, S) ----
    parts = [
        LAST.results[c]["o32"].reshape(O, SL, B).transpose(2, 0, 1)
        for c in range(NCORES)
    ]
    return np.ascontiguousarray(np.concatenate(parts, axis=2)).astype(np.int32)


# revision 6
# speedup vs baseline: 1.0080x; 1.0080x over previous
"""Trainium2 Bass kernel for nn_BitLayer.

Reference computation:
    x: (B=32, D=512, 1, S=64) int32 bits {0,1}
    kernel: (D=512, O=128, S=64) int32 bits {0,1}
    out[b, o, s] = (sum_d x[b,d,0,s] & kernel[d,o,s]) > 0     -> int32

Since the values are bits, AND == multiply, so for each bit position s this
is a (B x D) @ (D x O) matmul followed by a >0 threshold. The 64 bit
positions are fully independent, so we shard S across the 8 cores (8 bit
positions per core) — both inputs and the output shard along S, no
collectives needed.

Per core (S_loc = 8 bit positions):
  - host casts the {0,1} int32 bits to fp8_e4m3 (exact) and lays them out as
      k8: [128, S_loc*4*128] fp8   k8[p, (s*4+ch)*128 + o] = kernel[ch*128+p, o, s]
      x8: [128, S_loc*4*32]  fp8   x8[p, (s*4+ch)*32  + b] = x[b, ch*128+p, 0, s]
  - for each s: 4 accumulating PE matmuls over the D=512 contraction
      psum[o, b] += k8_chunk.T @ x8_chunk   (fp32 accumulate, sums <= 512: exact)
  - DVE threshold: out[o, s*32+b] = (psum > 0) as int32
  - one DMA out: o32 [128, S_loc*32] int32
"""

import numpy as np
import ml_dtypes

B, D, O, S = 32, 512, 128, 64
NCORES = 8
SL = S // NCORES          # bit positions per core = 8
P = 128                   # partition dim / contraction tile
CH = D // P               # contraction chunks = 4
F8NP = ml_dtypes.float8_e4m3

TRACE = False             # test harness can flip this for profiling
LAST = None               # last BassKernelResults (for the test harness)
IMPL = "raw"              # "raw" (manual sems, no Tile tail barrier) or "tile"

_NC = None                # cached Bass module (compile once)
_NC_IMPL = None


def _build():
    import concourse.mybir as mybir
    import concourse.tile as tile
    from concourse import bacc

    nc = bacc.Bacc(None, target_bir_lowering=False)
    f8 = mybir.dt.float8e4

    xd = nc.dram_tensor("x8", [P, SL * CH * B], f8, kind="ExternalInput")
    kd = nc.dram_tensor("k8", [P, SL * CH * O], f8, kind="ExternalInput")
    od = nc.dram_tensor("o32", [P, SL * B], mybir.dt.int32, kind="ExternalOutput")

    with tile.TileContext(nc) as tc:
        with (
            tc.tile_pool(name="inp", bufs=1) as pool,
            tc.tile_pool(name="ps", bufs=4, space="PSUM") as psum,
            tc.tile_pool(name="outp", bufs=1) as opool,
        ):
            xt = pool.tile([P, SL * CH * B], f8)
            kt = pool.tile([P, SL * CH * O], f8)
            nc.sync.dma_start(xt[:], xd[:])
            nc.sync.dma_start(kt[:], kd[:])

            ot = opool.tile([P, SL * B], mybir.dt.int32)
            for s in range(SL):
                ps = psum.tile([P, B], mybir.dt.float32)
                for ch in range(CH):
                    i = s * CH + ch
                    nc.tensor.matmul(
                        ps[:],
                        kt[:, i * O:(i + 1) * O],   # stationary lhsT [d, o]
                        xt[:, i * B:(i + 1) * B],   # moving rhs   [d, b]
                        start=(ch == 0),
                        stop=(ch == CH - 1),
                    )
                nc.vector.tensor_scalar(
                    ot[:, s * B:(s + 1) * B], ps[:], 0.0, None,
                    mybir.AluOpType.is_gt,
                )
            nc.sync.dma_start(od[:], ot[:])
    nc.compile()
    return nc


def _build_raw():
    from contextlib import ExitStack

    import concourse.mybir as mybir
    from concourse import bacc

    nc = bacc.Bacc(None, target_bir_lowering=False)
    f8 = mybir.dt.float8e4

    xd = nc.dram_tensor("x8", [P, SL * CH * B], f8, kind="ExternalInput")
    kd = nc.dram_tensor("k8", [P, SL * CH * O], f8, kind="ExternalInput")
    od = nc.dram_tensor("o32", [P, SL * B], mybir.dt.int32, kind="ExternalOutput")

    with ExitStack() as ctx:
        xt = ctx.enter_context(nc.sbuf_tensor("xt", [P, SL * CH * B], f8))
        kt = ctx.enter_context(nc.sbuf_tensor("kt", [P, SL * CH * O], f8))
        ot = ctx.enter_context(nc.sbuf_tensor("ot", [P, SL * B], mybir.dt.int32))
        pss = [
            ctx.enter_context(nc.psum_tensor(f"ps{s}", [P, B], mybir.dt.float32))
            for s in range(SL)
        ]
        dx = ctx.enter_context(nc.semaphore("dx"))
        dk = ctx.enter_context(nc.semaphore("dk"))
        pe = ctx.enter_context(nc.semaphore("pe"))
        dv = ctx.enter_context(nc.semaphore("dv"))
        do = ctx.enter_context(nc.semaphore("do"))

        with nc.Block() as block:

            @block.sync
            def _(sync):
                sync.dma_start(xt[:], xd[:]).then_inc(dx, 16)
                sync.dma_start(kt[:], kd[:]).then_inc(dk, 16)
                sync.wait_ge(dv, SL)
                sync.dma_start(od[:], ot[:]).then_inc(do, 16)
                sync.wait_ge(do, 16)

            @block.tensor
            def _(tensor):
                tensor.wait_ge(dx, 16)
                tensor.wait_ge(dk, 16)
                for s in range(SL):
                    mm = None
                    for ch in range(CH):
                        i = s * CH + ch
                        mm = tensor.matmul(
                            pss[s][:],
                            kt[:, i * O:(i + 1) * O],
                            xt[:, i * B:(i + 1) * B],
                            start=(ch == 0),
                            stop=(ch == CH - 1),
                        )
                    mm.then_inc(pe, 1)

            @block.vector
            def _(vector):
                for s in range(SL):
                    vector.wait_ge(pe, s + 1)
                    vector.tensor_scalar(
                        ot[:, s * B:(s + 1) * B], pss[s][:], 0.0, None,
                        mybir.AluOpType.is_gt,
                    ).then_inc(dv, 1)

    nc.compile()
    return nc


def kernel(x: np.ndarray, kernel: np.ndarray) -> np.ndarray:
    global _NC, _NC_IMPL, LAST
    from concourse.bass_utils import run_bass_kernel_spmd

    if _NC is None or _NC_IMPL != IMPL:
        _NC = _build_raw() if IMPL == "raw" else _build()
        _NC_IMPL = IMPL

    # ---- host-side shard + layout (values are {0,1}: fp8 cast is exact) ----
    # x: (B, D, 1, S) -> (S, D, B) -> per core [128, SL*CH*B]
    xr = np.ascontiguousarray(
        x.reshape(B, D, S).astype(F8NP).transpose(2, 1, 0)
    ).reshape(NCORES, SL, CH, P, B).transpose(0, 3, 1, 2, 4)
    # kernel: (D, O, S) -> (S, D, O) -> per core [128, SL*CH*O]
    kr = np.ascontiguousarray(
        kernel.astype(F8NP).transpose(2, 0, 1)
    ).reshape(NCORES, SL, CH, P, O).transpose(0, 3, 1, 2, 4)

    in_maps = [
        {
            "x8": np.ascontiguousarray(xr[c]).reshape(P, SL * CH * B),
            "k8": np.ascontiguousarray(kr[c]).reshape(P, SL * CH * O),
        }
        for c in range(NCORES)
    ]

    LAST = run_bass_kernel_spmd(
        _NC, in_maps, core_ids=list(range(NCORES)), trace=TRACE
    )

    # ---- gather: per-core o32 [128, SL*32] = (o, s, b) -> (B, O, S) ----
    parts = [
        LAST.results[c]["o32"].reshape(O, SL, B).transpose(2, 0, 1)
        for c in range(NCORES)
    ]
    return np.ascontiguousarray(np.concatenate(parts, axis=2)).astype(np.int32)


# revision 9
# speedup vs baseline: 1.2326x; 1.2228x over previous
"""Trainium2 Bass kernel for nn_BitLayer.

Reference computation:
    x: (B=32, D=512, 1, S=64) int32 bits {0,1}
    kernel: (D=512, O=128, S=64) int32 bits {0,1}
    out[b, o, s] = (sum_d x[b,d,0,s] & kernel[d,o,s]) > 0     -> int32

Since the values are bits, AND == multiply, so for each bit position s this
is a (B x D) @ (D x O) matmul followed by a >0 threshold. The 64 bit
positions are fully independent, so we shard S across the 8 cores (8 bit
positions per core) — both inputs and the output shard along S, no
collectives needed.

Per core (S_loc = 8 bit positions):
  - host casts the {0,1} int32 bits to fp8_e4m3 (exact) and lays them out as
      k8: [128, S_loc*4*128] fp8   k8[p, (s*4+ch)*128 + o] = kernel[ch*128+p, o, s]
      x8: [128, S_loc*4*32]  fp8   x8[p, (s*4+ch)*32  + b] = x[b, ch*128+p, 0, s]
  - for each s: 4 accumulating PE matmuls over the D=512 contraction
      psum[o, b] += k8_chunk.T @ x8_chunk   (fp32 accumulate, sums <= 512: exact)
  - DVE threshold: out[o, s*32+b] = (psum > 0) as int32
  - one DMA out: o32 [128, S_loc*32] int32
"""

import numpy as np
import ml_dtypes

B, D, O, S = 32, 512, 128, 64
NCORES = 8
SL = S // NCORES          # bit positions per core = 8
P = 128                   # partition dim / contraction tile
CH = D // P               # contraction chunks = 4
F8NP = ml_dtypes.float8_e4m3

TRACE = False             # test harness can flip this for profiling
LAST = None               # last BassKernelResults (for the test harness)
IMPL = "raw2"             # "raw2" | "raw" | "tile"

_NC = None                # cached Bass module (compile once)
_NC_IMPL = None


def _build():
    import concourse.mybir as mybir
    import concourse.tile as tile
    from concourse import bacc

    nc = bacc.Bacc(None, target_bir_lowering=False)
    f8 = mybir.dt.float8e4

    xd = nc.dram_tensor("x8", [P, SL * CH * B], f8, kind="ExternalInput")
    kd = nc.dram_tensor("k8", [P, SL * CH * O], f8, kind="ExternalInput")
    od = nc.dram_tensor("o32", [P, SL * B], mybir.dt.int32, kind="ExternalOutput")

    with tile.TileContext(nc) as tc:
        with (
            tc.tile_pool(name="inp", bufs=1) as pool,
            tc.tile_pool(name="ps", bufs=4, space="PSUM") as psum,
            tc.tile_pool(name="outp", bufs=1) as opool,
        ):
            xt = pool.tile([P, SL * CH * B], f8)
            kt = pool.tile([P, SL * CH * O], f8)
            nc.sync.dma_start(xt[:], xd[:])
            nc.sync.dma_start(kt[:], kd[:])

            ot = opool.tile([P, SL * B], mybir.dt.int32)
            for s in range(SL):
                ps = psum.tile([P, B], mybir.dt.float32)
                for ch in range(CH):
                    i = s * CH + ch
                    nc.tensor.matmul(
                        ps[:],
                        kt[:, i * O:(i + 1) * O],   # stationary lhsT [d, o]
                        xt[:, i * B:(i + 1) * B],   # moving rhs   [d, b]
                        start=(ch == 0),
                        stop=(ch == CH - 1),
                    )
                nc.vector.tensor_scalar(
                    ot[:, s * B:(s + 1) * B], ps[:], 0.0, None,
                    mybir.AluOpType.is_gt,
                )
            nc.sync.dma_start(od[:], ot[:])
    nc.compile()
    return nc


def _build_raw():
    from contextlib import ExitStack

    import concourse.mybir as mybir
    from concourse import bacc

    nc = bacc.Bacc(None, target_bir_lowering=False)
    f8 = mybir.dt.float8e4

    xd = nc.dram_tensor("x8", [P, SL * CH * B], f8, kind="ExternalInput")
    kd = nc.dram_tensor("k8", [P, SL * CH * O], f8, kind="ExternalInput")
    od = nc.dram_tensor("o32", [P, SL * B], mybir.dt.int32, kind="ExternalOutput")

    with ExitStack() as ctx:
        xt = ctx.enter_context(nc.sbuf_tensor("xt", [P, SL * CH * B], f8))
        kt = ctx.enter_context(nc.sbuf_tensor("kt", [P, SL * CH * O], f8))
        ot = ctx.enter_context(nc.sbuf_tensor("ot", [P, SL * B], mybir.dt.int32))
        pss = [
            ctx.enter_context(nc.psum_tensor(f"ps{s}", [P, B], mybir.dt.float32))
            for s in range(SL)
        ]
        dx = ctx.enter_context(nc.semaphore("dx"))
        dk = ctx.enter_context(nc.semaphore("dk"))
        pe = ctx.enter_context(nc.semaphore("pe"))
        dv = ctx.enter_context(nc.semaphore("dv"))
        do = ctx.enter_context(nc.semaphore("do"))

        with nc.Block() as block:

            @block.sync
            def _(sync):
                sync.dma_start(xt[:], xd[:]).then_inc(dx, 16)
                sync.dma_start(kt[:], kd[:]).then_inc(dk, 16)
                sync.wait_ge(dv, SL)
                sync.dma_start(od[:], ot[:]).then_inc(do, 16)
                sync.wait_ge(do, 16)

            @block.tensor
            def _(tensor):
                tensor.wait_ge(dx, 16)
                tensor.wait_ge(dk, 16)
                for s in range(SL):
                    mm = None
                    for ch in range(CH):
                        i = s * CH + ch
                        mm = tensor.matmul(
                            pss[s][:],
                            kt[:, i * O:(i + 1) * O],
                            xt[:, i * B:(i + 1) * B],
                            start=(ch == 0),
                            stop=(ch == CH - 1),
                        )
                    mm.then_inc(pe, 1)

            @block.vector
            def _(vector):
                for s in range(SL):
                    vector.wait_ge(pe, s + 1)
                    vector.tensor_scalar(
                        ot[:, s * B:(s + 1) * B], pss[s][:], 0.0, None,
                        mybir.AluOpType.is_gt,
                    ).then_inc(dv, 1)

    nc.compile()
    return nc


def _build_raw2():
    """No-Block variant: per-engine streams emitted directly, no Block-exit
    all-engine barrier. The framework epilogue's per-engine DRAINs guarantee
    the final output DMA lands before the NEFF completes, so its ~2us
    completion latency overlaps the (fixed) epilogue tail instead of
    serializing in front of it. Input DMAs are split across both HWDGE rings
    (Sync + Scalar) and the kernel tensor is chunked so the PE can start
    after the first half arrives."""
    from contextlib import ExitStack

    import concourse.mybir as mybir
    from concourse import bacc

    nc = bacc.Bacc(None, target_bir_lowering=False)
    f8 = mybir.dt.float8e4
    HALF = SL // 2

    xd = nc.dram_tensor("x8", [P, SL * CH * B], f8, kind="ExternalInput")
    kd = nc.dram_tensor("k8", [P, SL * CH * O], f8, kind="ExternalInput")
    od = nc.dram_tensor("o8", [P, SL * B], mybir.dt.uint8, kind="ExternalOutput")

    K0 = HALF * CH * O  # free-dim split point of the kernel tensor

    with ExitStack() as ctx:
        xt = ctx.enter_context(nc.sbuf_tensor("xt", [P, SL * CH * B], f8))
        kt = ctx.enter_context(nc.sbuf_tensor("kt", [P, SL * CH * O], f8))
        ot = ctx.enter_context(nc.sbuf_tensor("ot", [P, SL * B], mybir.dt.uint8))
        pss = [
            ctx.enter_context(nc.psum_tensor(f"ps{s}", [P, B], mybir.dt.float32))
            for s in range(SL)
        ]
        dx = nc.alloc_semaphore("dx")
        dk0 = nc.alloc_semaphore("dk0")
        dk1 = nc.alloc_semaphore("dk1")
        pe = nc.alloc_semaphore("pe")
        dv = nc.alloc_semaphore("dv")
        do = nc.alloc_semaphore("do")

        # Sync ring: first kernel half (PE's first dependency alongside x).
        nc.sync.dma_start(kt[:, :K0], kd[:, :K0]).then_inc(dk0, 16)
        # Scalar ring (runs concurrently): x, then second kernel half.
        nc.scalar.dma_start(xt[:], xd[:]).then_inc(dx, 16)
        nc.scalar.dma_start(kt[:, K0:], kd[:, K0:]).then_inc(dk1, 16)

        # TensorE: 4 accumulating matmuls per bit position.
        nc.tensor.wait_ge(dx, 16)
        nc.tensor.wait_ge(dk0, 16)
        for s in range(SL):
            if s == HALF:
                nc.tensor.wait_ge(dk1, 16)
            mm = None
            for ch in range(CH):
                i = s * CH + ch
                mm = nc.tensor.matmul(
                    pss[s][:],
                    kt[:, i * O:(i + 1) * O],
                    xt[:, i * B:(i + 1) * B],
                    start=(ch == 0),
                    stop=(ch == CH - 1),
                )
            mm.then_inc(pe, 1)

        # DVE: threshold each psum group as it completes.
        for s in range(SL):
            nc.vector.wait_ge(pe, s + 1)
            nc.vector.tensor_scalar(
                ot[:, s * B:(s + 1) * B], pss[s][:], 0.0, None,
                mybir.AluOpType.is_gt,
            ).then_inc(dv, 1)

        # Sync: ship the result. No completion wait — the framework
        # epilogue's Sync DRAIN retires the queue before the NEFF ends.
        nc.sync.wait_ge(dv, SL)
        nc.sync.dma_start(od[:], ot[:]).then_inc(do, 16)

    nc.compile()
    return nc


_BUILDERS = {"tile": _build, "raw": _build_raw, "raw2": _build_raw2}


def kernel(x: np.ndarray, kernel: np.ndarray) -> np.ndarray:
    global _NC, _NC_IMPL, LAST
    from concourse.bass_utils import run_bass_kernel_spmd

    if _NC is None or _NC_IMPL != IMPL:
        _NC = _BUILDERS[IMPL]()
        _NC_IMPL = IMPL

    # ---- host-side shard + layout (values are {0,1}: fp8 cast is exact) ----
    # x: (B, D, 1, S) -> (S, D, B) -> per core [128, SL*CH*B]
    xr = np.ascontiguousarray(
        x.reshape(B, D, S).astype(F8NP).transpose(2, 1, 0)
    ).reshape(NCORES, SL, CH, P, B).transpose(0, 3, 1, 2, 4)
    # kernel: (D, O, S) -> (S, D, O) -> per core [128, SL*CH*O]
    kr = np.ascontiguousarray(
        kernel.astype(F8NP).transpose(2, 0, 1)
    ).reshape(NCORES, SL, CH, P, O).transpose(0, 3, 1, 2, 4)

    in_maps = [
        {
            "x8": np.ascontiguousarray(xr[c]).reshape(P, SL * CH * B),
            "k8": np.ascontiguousarray(kr[c]).reshape(P, SL * CH * O),
        }
        for c in range(NCORES)
    ]

    LAST = run_bass_kernel_spmd(
        _NC, in_maps, core_ids=list(range(NCORES)), trace=TRACE
    )

    # ---- gather: per-core out [128, SL*32] = (o, s, b) -> (B, O, S) ----
    okey = next(iter(LAST.results[0]))
    parts = [
        LAST.results[c][okey].reshape(O, SL, B).transpose(2, 0, 1)
        for c in range(NCORES)
    ]
    return np.ascontiguousarray(np.concatenate(parts, axis=2)).astype(np.int32)
